# revision 11
# baseline (speedup 1.0000x reference)
"""Multi-head attention (B=16, S=1024, D=768, H=12) on 8 TRN2 NeuronCores.

Strategy: pure data parallelism — batch 16 is split 2-per-core; weights are
replicated. Each core runs an identical Bass/Tile program on its own x shard,
so no collectives are needed. Host-side marshaling pre-transposes x and the
weights into the d-major layouts the PE array contracts over.

All PE operands are bf16 (fp32 LDWEIGHTS streams 2x slower and fp32r matmuls
measure ~100ns/512col slower than bf16 on HW; bf16 end-to-end rel err ~8e-3
vs the 2e-2 budget). PSUM accumulation stays fp32 throughout.

Per-core program (b in 0..1, head-pairs hp in 0..5):
  - v  = x @ W_v^T           natural [t, e] layout, stored head-interleaved
                             with a ones column -> PV lhsT [k, 64+1] per head
  - qT2/kT2 [128, S]         two heads stacked on partitions (d-major, bf16)
  - scoresT[k,q] = k q^T     row-packed per head via tile_position (K=64)
  - exp on ACT (scale=1/8) -> bf16 SBUF tile
  - PV: out[dh+1, q] += v_ext.T @ exp   (row 64 accumulates the softmax denom)
  - normalize: one [65,512] copy to SBUF per accumulator (frees PSUM fast),
    per-row reciprocal_approx_fast on the denom row, gpsimd
    partition_broadcast, DVE mult -> attn_outT [d, t] (bf16)
  - y = attn_outT.T @ W_out^T   bias added on DVE from a broadcast bias tile
    (saves the K=1 bias matmuls on the PE)

The attention inner loop paces PE and ACT nearly evenly (~1.3us ACT vs
~1.9us PE per kt step incl. fills). To keep the in-order PE busy and the ACT
stream gapless across batch boundaries, vgen of the next batch and the
projection of the previous batch are emitted as "fill" work interleaved
between attention pipeline steps.
"""
import ml_dtypes
import numpy as np
import concourse.bacc as bacc
import concourse.tile as tile
from concourse import mybir
from concourse.bass_utils import run_bass_kernel_spmd

FP32 = mybir.dt.float32
BF16 = mybir.dt.bfloat16
MMDT = BF16                          # dtype for all PE operands
NPMM = ml_dtypes.bfloat16
EXP = mybir.ActivationFunctionType.Exp

B, S, D, H = 2, 1024, 768, 12       # per-core batch of 2
HP = H // 2                          # head pairs
DT = D // 128                        # d tiles (6)
KT = S // 128                        # k tiles (8)
QC = S // 512                        # q chunks (2)
TT = S // 128                        # t tiles per batch (8)
N_CORES = 8

_CACHE = {}


def build_nc():
    nc = bacc.Bacc(trn_type="TRN2")
    xT = nc.dram_tensor("xT", [D, B * S], MMDT, kind="ExternalInput")
    wqkvT = nc.dram_tensor("wqkvT", [D, 3 * D], MMDT, kind="ExternalInput")
    woutT = nc.dram_tensor("woutT", [D, D], BF16, kind="ExternalInput")
    bout = nc.dram_tensor("bout", [1, D], FP32, kind="ExternalInput")
    y = nc.dram_tensor("y", [B * S, D], FP32, kind="ExternalOutput")

    with tile.TileContext(nc) as tc:
        with (
            tc.tile_pool(name="wq", bufs=1) as p_wq,
            tc.tile_pool(name="wo", bufs=1) as p_wo,
            tc.tile_pool(name="cst", bufs=1) as p_cst,
            tc.tile_pool(name="xt", bufs=1) as p_xt,
            tc.tile_pool(name="vv", bufs=2) as p_v,
            tc.tile_pool(name="ao", bufs=2) as p_ao,
            tc.tile_pool(name="qk", bufs=4) as p_qk,
            tc.tile_pool(name="exp", bufs=3) as p_exp,
            tc.tile_pool(name="oc", bufs=6) as p_oc,
            tc.tile_pool(name="yy", bufs=2) as p_y,
            tc.tile_pool(name="rb", bufs=2) as p_rb,
            tc.tile_pool(name="r0", bufs=4) as p_r0,
            tc.tile_pool(name="sc", bufs=2, space="PSUM") as p_sc,
            tc.tile_pool(name="gen", bufs=2, space="PSUM") as p_gen,
            tc.tile_pool(name="oacc", bufs=2, space="PSUM") as p_oacc,
        ):
            wq = p_wq.tile([128, DT, 3 * D], MMDT)
            wo = p_wo.tile([128, DT, D], BF16)
            bo = p_cst.tile([1, D], FP32)
            bias_bc = p_cst.tile([128, D], FP32)
            nc.sync.dma_start(bo[:], bout[:])
            nc.gpsimd.partition_broadcast(bias_bc[:], bo[:])
            for j in range(DT):
                nc.sync.dma_start(
                    wq[:, j, 2 * D:3 * D],
                    wqkvT[128 * j:128 * (j + 1), 2 * D:3 * D],
                )
            for j in range(DT):  # head-pair 0 q/k columns first
                nc.sync.dma_start(
                    wq[:, j, 0:128], wqkvT[128 * j:128 * (j + 1), 0:128]
                )
                nc.sync.dma_start(
                    wq[:, j, D:D + 128], wqkvT[128 * j:128 * (j + 1), D:D + 128]
                )
            def load_wq_rest():
                for j in range(DT):
                    nc.sync.dma_start(
                        wq[:, j, 128:D], wqkvT[128 * j:128 * (j + 1), 128:D]
                    )
                    nc.sync.dma_start(
                        wq[:, j, D + 128:2 * D],
                        wqkvT[128 * j:128 * (j + 1), D + 128:2 * D],
                    )
                    nc.sync.dma_start(wo[:, j, :], woutT[128 * j:128 * (j + 1), :])

            xts, vs, aos = {}, {}, {}

            def load_xt(b):
                # qc0 halves first so qkgen/vgen for the first 512 tokens can
                # start while the second half is still in flight
                xt = p_xt.tile([128, DT, S], MMDT, tag="xt")
                for half in range(2):
                    for j in range(DT):
                        nc.sync.dma_start(
                            xt[:, j, half * 512:(half + 1) * 512],
                            xT[128 * j:128 * (j + 1),
                               b * S + half * 512:b * S + (half + 1) * 512],
                        )
                xts[b] = xt

            def alloc_v(b):
                v = p_v.tile([128, KT, H, 65], BF16, tag="vv")
                nc.vector.memset(v[:, :, :, 64], 1.0)
                vs[b] = v

            def vgen_fills(b):
                """16 closures: one [128,512-or-256] psum group + copy each."""
                fills = []
                for tt in range(TT):
                    for h0, nh in ((0, 8), (8, 4)):
                        def f(tt=tt, h0=h0, nh=nh, b=b):
                            xt, v = xts[b], vs[b]
                            vp = p_gen.tile([128, 512], FP32, tag="gen")
                            cw = nh * 64
                            for j in range(DT):
                                nc.tensor.matmul(
                                    vp[:, 0:cw],
                                    xt[:, j, tt * 128:(tt + 1) * 128],
                                    wq[:, j,
                                       2 * D + h0 * 64:2 * D + h0 * 64 + cw],
                                    start=(j == 0), stop=(j == DT - 1),
                                )
                            nc.vector.tensor_copy(
                                v[:, tt, h0:h0 + nh, 0:64],
                                vp[:, 0:cw].rearrange("p (h c) -> p h c", h=nh),
                            )
                        fills.append(f)
                return fills

            def proj_fills(b):
                """16 closures: y(b) projection, one psum chunk-group each."""
                fills = []
                for tt in range(TT):
                    box = {}
                    for ci, (c0, cw) in enumerate(((0, 512), (512, 256))):
                        def f(tt=tt, ci=ci, c0=c0, cw=cw, b=b, box=box):
                            ao = aos[b]
                            if ci == 0:
                                ys = p_y.tile([128, D], FP32, tag="yy")
                                box["ys"] = ys
                            ys = box["ys"]
                            yp = p_gen.tile([128, 512], FP32, tag="gen")
                            for j in range(DT):
                                nc.tensor.matmul(
                                    yp[:, 0:cw],
                                    ao[:, j, tt * 128:(tt + 1) * 128],
                                    wo[:, j, c0:c0 + cw],
                                    start=(j == 0), stop=(j == DT - 1),
                                )
                            nc.vector.tensor_add(
                                ys[:, c0:c0 + cw], yp[:, 0:cw],
                                bias_bc[:, c0:c0 + cw],
                            )
                            if ci == 1:
                                nc.sync.dma_start(
                                    y[b * S + tt * 128:b * S + (tt + 1) * 128, :],
                                    ys[:],
                                )
                        fills.append(f)
                return fills

            def make_qk(b, hp):
                """Allocate the unit's q/k tiles and return 4 fill closures
                that generate them (run inside the PREVIOUS unit so the next
                unit's scores can start immediately)."""
                sq_q = p_qk.tile([128, S], MMDT, tag="qk")
                sq_k = p_qk.tile([128, S], MMDT, tag="qk")
                qkt = [sq_q, sq_k]

                def f(part, qc):
                    def g():
                        qp = p_gen.tile([128, 512], FP32, tag="gen")
                        for j in range(DT):
                            nc.tensor.matmul(
                                qp[:, :],
                                wq[:, j,
                                   part * D + 128 * hp:part * D + 128 * (hp + 1)],
                                xts[b][:, j, qc * 512:(qc + 1) * 512],
                                start=(j == 0), stop=(j == DT - 1),
                            )
                        nc.vector.tensor_copy(
                            qkt[part][:, qc * 512:(qc + 1) * 512], qp[:, :]
                        )
                    return g

                fills = [f(p, q) for p in range(2) for q in range(QC)]
                return qkt, fills

            def sched_fills(*streams):
                """streams: (start_step, per_step, closures). Returns
                (by_step[16], tail) with closures placed in order."""
                by_step = [[] for _ in range(16)]
                tail = []
                for start, per, closures in streams:
                    q = list(closures)
                    for s in range(start, 16):
                        for _ in range(per):
                            if q:
                                by_step[s].append(q.pop(0))
                    tail.extend(q)
                return by_step, tail

            def unit(b, hp, qkt, by_step, tail=(), early_fills=()):
                """attention unit on pre-generated q/k (fill work interleaved
                per by_step); returns the qc1-half normalize closure (qc0's
                runs inside the unit); the caller defers it past the next
                unit's start."""
                v, ao = vs[b], aos[b]
                qT2, kT2 = qkt

                ocs, oaccs = {}, {}

                def scores_exp(qc, kt):
                    sc = p_sc.tile([128, 1024], FP32, tag="sc")
                    nc.tensor.matmul(
                        sc[:, 0:512],
                        kT2[0:64, kt * 128:(kt + 1) * 128],
                        qT2[0:64, qc * 512:(qc + 1) * 512],
                        start=True, stop=True, tile_position=(0, 0),
                    )
                    nc.tensor.matmul(
                        sc[:, 512:1024],
                        kT2[64:128, kt * 128:(kt + 1) * 128],
                        qT2[64:128, qc * 512:(qc + 1) * 512],
                        start=True, stop=True, tile_position=(64, 0),
                    )
                    ex = p_exp.tile([128, 1024], BF16, tag="exp")
                    nc.scalar.activation(ex[:], sc[:], EXP, scale=0.125)
                    return ex

                def pv(qc, kt, ex):
                    if kt == 0:
                        o_a = p_oacc.tile([65, 512], FP32, tag="oacc")
                        o_b = p_oacc.tile([65, 512], FP32, tag="oacc")
                        oaccs[(qc, 0)] = o_a
                        oaccs[(qc, 1)] = o_b
                    nc.tensor.matmul(
                        oaccs[(qc, 0)][:], v[:, kt, 2 * hp, :], ex[:, 0:512],
                        start=(kt == 0), stop=(kt == KT - 1),
                    )
                    nc.tensor.matmul(
                        oaccs[(qc, 1)][:], v[:, kt, 2 * hp + 1, :],
                        ex[:, 512:1024],
                        start=(kt == 0), stop=(kt == KT - 1),
                    )
                    if kt == KT - 1:
                        # one copy frees the PSUM accumulator (row 64 = denom)
                        for head in range(2):
                            oc = p_oc.tile([65, 512], FP32, tag="oc")
                            nc.vector.tensor_copy(oc[:], oaccs[(qc, head)][:])
                            ocs[(qc, head)] = oc

                def norm_qc(qc):
                    for head in range(2):
                        oc = ocs[(qc, head)]
                        # reciprocal_approx_fast and partition_broadcast
                        # only honor base-partition-0 inputs on HW; shift
                        # the denom row down first
                        r0 = p_r0.tile([1, 512], FP32, tag="r0")
                        nc.vector.tensor_copy(r0[:], oc[64:65, :])
                        rr = p_r0.tile([1, 512], FP32, tag="rr")
                        nc.vector.reciprocal_approx_fast(rr[:], r0[:])
                        rb = p_rb.tile([64, 512], FP32, tag="rb")
                        nc.gpsimd.partition_broadcast(rb[:], rr[:])
                        nc.vector.tensor_mul(
                            ao[64 * head:64 * (head + 1), hp,
                               qc * 512:(qc + 1) * 512],
                            oc[0:64, :], rb[:],
                        )

                # 16-step pipeline: scores/exp one step ahead of PV; fill
                # work drips in between steps. qc0's normalize runs inside
                # the unit (ocs ready after step 8); qc1's is returned for
                # deferral past the next unit's start.
                eq = list(early_fills)
                prev = None
                step = 0
                for qc in range(QC):
                    for kt in range(KT):
                        ex = scores_exp(qc, kt)
                        if prev is not None:
                            pv(*prev)
                        prev = (qc, kt, ex)
                        if eq:
                            eq.pop(0)()
                        if step == 11:
                            norm_qc(0)
                        for f in by_step[step]:
                            f()
                        step += 1
                pv(*prev)
                for f in eq:
                    f()
                for f in tail:
                    f()

                return lambda: norm_qc(1)

            # ---- schedule ----
            load_xt(0)
            alloc_v(0)
            load_wq_rest()
            qkt, qk_fills = make_qk(0, 0)
            for f in qk_fills:  # first unit's q/k generated inline
                f()
            norm = None
            for b in range(B):
                ao = p_ao.tile([128, DT, S], BF16, tag="ao")
                aos[b] = ao
                for hp in range(HP):
                    early = [norm] if norm is not None else []
                    last = (hp == HP - 1 and b == B - 1)
                    if not last:
                        nb, nhp = (b, hp + 1) if hp < HP - 1 else (b + 1, 0)
                        if nhp == 0:
                            # stage next batch's x (vgen/qk fills below need
                            # a few steps of DMA headroom)
                            load_xt(b + 1)
                            alloc_v(b + 1)
                        next_qkt, nqf = make_qk(nb, nhp)
                    if hp == HP - 1 and b + 1 < B:
                        # next batch: its qk fills (need xt qc0 half, ~4
                        # steps of headroom) then its vgen interleaved
                        by_step, tl = sched_fills((4, 1, nqf),
                                                  (7, 2, vgen_fills(b + 1)))
                    elif hp == 0 and b == 0:
                        # this batch's own vgen fills the first unit at
                        # 2/step so each v[tt] lands just ahead of the PV
                        # that consumes it; next unit's qk after
                        by_step, tl = sched_fills((0, 2, vgen_fills(0)),
                                                  (8, 2, nqf))
                    elif hp == 0 and b > 0:
                        # previous batch's projection fills this unit; the
                        # deferred normalize of unit (b-1, 5) must land
                        # before any projection group reads its ao band
                        by_step, tl = sched_fills((1, 2, proj_fills(b - 1)),
                                                  (9, 1, nqf))
                    elif not last:
                        by_step, tl = sched_fills((1, 1, nqf))
                    else:
                        by_step, tl = sched_fills()
                    norm = unit(b, hp, qkt, by_step, tail=tl,
                                early_fills=early)
                    if not last:
                        qkt = next_qkt
            norm()
            for f in proj_fills(B - 1):
                f()
    nc.finalize()
    return nc


def _marshal(x, W_qkv, W_out, b_out):
    wqkvT = np.ascontiguousarray(W_qkv.T).astype(NPMM)
    woutT = np.ascontiguousarray(W_out.T).astype(ml_dtypes.bfloat16)
    bo = np.ascontiguousarray(b_out.reshape(1, D)).astype(np.float32)
    in_maps = []
    for c in range(N_CORES):
        xc = np.ascontiguousarray(
            np.asarray(x)[B * c:B * (c + 1)].reshape(B * S, D).T
        ).astype(NPMM)
        in_maps.append({
            "xT": xc, "wqkvT": wqkvT, "woutT": woutT, "bout": bo,
        })
    return in_maps


def run(x, W_qkv, W_out, b_out, trace=False, **spmd_kwargs):
    if "nc" not in _CACHE:
        _CACHE["nc"] = build_nc()
    nc = _CACHE["nc"]
    in_maps = _marshal(x, W_qkv, W_out, b_out)
    res = run_bass_kernel_spmd(
        nc, in_maps, core_ids=list(range(N_CORES)), trace=trace, **spmd_kwargs
    )
    out = np.stack([res.results[c]["y"] for c in range(N_CORES)], axis=0)
    out = out.reshape(N_CORES * B, S, D)
    return out, res


def kernel(x, W_qkv, W_out, b_out):
    out, _ = run(x, W_qkv, W_out, b_out)
    return out
